# revision 4
# baseline (speedup 1.0000x reference)
"""AdaptiveDiffusionLayer on 8 TRN2 NeuronCores.

out = (1 - t) * support + t * (adj @ support),  support = x @ weight

Strategy (1D row-parallel SpMM):
  - Row-shard adj and x across 8 cores (1250 rows each); replicate weight/t.
  - Host-side: pre-transpose + bf16-cast each core's adj shard so the
    contraction index k lands on the SBUF partition axis with unit-stride
    DMA (no on-device transposes), packed as [2, N, 625] so each of the
    two PSUM passes reads contiguous slabs.
  - Device: support_c = x_c @ W (bf16 matmul, fp32 PSUM) -> AllGather of
    bf16 support via DRAM bounce -> adj_c @ support accumulated over 79
    k-tiles into 5 PSUM banks per pass (2 passes x 5 i-subtiles of 125
    rows) -> fused epilogue (t * acc + (1-t) * support_c) -> out.
"""

import sys

for _p in ("/opt/trn_rl_repo",):
    if _p not in sys.path:
        sys.path.append(_p)

import numpy as np
import ml_dtypes

from concourse import bass, bacc, mybir, tile
from concourse.bass_utils import run_bass_kernel_spmd

N = 10000
IN_F = 512
OUT_F = 512
C = 8            # cores
R = N // C       # 1250 rows per core
HALF = R // 2    # 625, i-columns per pass
NSUB = 5         # i-subtiles per pass
SUB = HALF // NSUB  # 125 rows per i-subtile
KT = (N + 127) // 128   # 79 k-tiles
KLAST = N - (KT - 1) * 128  # 16

BF16 = mybir.dt.bfloat16
F32 = mybir.dt.float32

_cached = {}


def _build():
    nc = bacc.Bacc("TRN2", target_bir_lowering=False, debug=False, num_devices=C)

    adjt = nc.dram_tensor("adjt", [2, N, HALF], BF16, kind="ExternalInput")
    xt = nc.dram_tensor("xt", [IN_F, R], BF16, kind="ExternalInput")
    w = nc.dram_tensor("w", [IN_F, OUT_F], BF16, kind="ExternalInput")
    tsc = nc.dram_tensor("tsc", [128, 2], F32, kind="ExternalInput")
    out = nc.dram_tensor("out", [R, OUT_F], F32, kind="ExternalOutput")

    sup_in = nc.dram_tensor("sup_in", [R, OUT_F], BF16)
    sup_gath = nc.dram_tensor("sup_gath", [N, OUT_F], BF16, addr_space="Shared")

    NJ = IN_F // 128  # 4 contraction tiles for x @ W

    with tile.TileContext(nc) as tc:
        with (
            tc.tile_pool(name="persist", bufs=1) as p_pers,
            tc.tile_pool(name="psum_sup", bufs=2, space="PSUM") as pp_sup,
            tc.tile_pool(name="supbf_pool", bufs=3) as p_supbf,
            tc.tile_pool(name="supsc_pool", bufs=1) as p_supsc,
            tc.tile_pool(name="sup_pool", bufs=1) as p_sup,
            tc.tile_pool(name="slab_pool", bufs=4) as p_slab,
            tc.tile_pool(name="psum_main", bufs=1, space="PSUM") as pp_main,
            tc.tile_pool(name="out_pool", bufs=4) as p_out,
        ):
            xt_sb = p_pers.tile([128, NJ * R], BF16, tag="xt_sb", name="xt_sb")
            w_sb = p_pers.tile([128, NJ * OUT_F], BF16, tag="w_sb", name="w_sb")
            tsc_sb = p_pers.tile([128, 2], F32, tag="tsc_sb", name="tsc_sb")

            for j in range(NJ):
                nc.scalar.dma_start(
                    out=xt_sb[:, j * R:(j + 1) * R],
                    in_=xt[j * 128:(j + 1) * 128, :],
                )
                nc.scalar.dma_start(
                    out=w_sb[:, j * OUT_F:(j + 1) * OUT_F],
                    in_=w[j * 128:(j + 1) * 128, :],
                )
            nc.scalar.dma_start(out=tsc_sb[:, :], in_=tsc[:, :])

            # ---- support_c = x_c @ W, 10 i-subtiles of 125 rows ----
            supsc = []
            for s in range(2 * NSUB):
                ps = pp_sup.tile([SUB, OUT_F], F32, tag="ps", name=f"ps{s}")
                for j in range(NJ):
                    nc.tensor.matmul(
                        ps[:, :],
                        lhsT=xt_sb[:, j * R + s * SUB: j * R + (s + 1) * SUB],
                        rhs=w_sb[:, j * OUT_F:(j + 1) * OUT_F],
                        start=(j == 0),
                        stop=(j == NJ - 1),
                    )
                sb = p_supbf.tile([SUB, OUT_F], BF16, tag="supbf", name=f"supbf{s}")
                nc.vector.tensor_copy(sb[:, :], ps[:, :])
                sc = p_supsc.tile([SUB, OUT_F], F32, tag=f"sc{s}", name=f"sc{s}")
                nc.vector.tensor_scalar_mul(sc[:, :], ps[:, :], tsc_sb[0:SUB, 1:2])
                supsc.append(sc)
                nc.gpsimd.dma_start(
                    out=sup_in[s * SUB:(s + 1) * SUB, :], in_=sb[:, :]
                )

            # ---- AllGather bf16 support across the 8 cores ----
            nc.gpsimd.collective_compute(
                "AllGather",
                mybir.AluOpType.bypass,
                replica_groups=[list(range(C))],
                ins=[sup_in.ap().opt()],
                outs=[sup_gath.ap().opt()],
            )

            # ---- load gathered support as [k-part, f] tiles ----
            sup_tiles = []
            for kk in range(KT):
                sz = 128 if kk < KT - 1 else KLAST
                stile = p_sup.tile([sz, OUT_F], BF16, tag=f"sup{kk}", name=f"sup{kk}")
                nc.scalar.dma_start(
                    out=stile[:, :], in_=sup_gath[kk * 128:kk * 128 + sz, :]
                )
                sup_tiles.append(stile)

            # ---- main SpMM: 2 passes x 5 PSUM accumulators over 79 k-tiles ----
            for p in range(2):
                acc = [
                    pp_main.tile(
                        [SUB, OUT_F], F32, tag=f"acc{s}", name=f"acc{p}_{s}"
                    )
                    for s in range(NSUB)
                ]
                for kk in range(KT):
                    sz = 128 if kk < KT - 1 else KLAST
                    slab = p_slab.tile(
                        [128, HALF], BF16, tag="slab", name=f"slab{p}_{kk}"
                    )
                    nc.sync.dma_start(
                        out=slab[0:sz, :], in_=adjt[p, kk * 128:kk * 128 + sz, :]
                    )
                    for s in range(NSUB):
                        nc.tensor.matmul(
                            acc[s][:, :],
                            lhsT=slab[0:sz, s * SUB:(s + 1) * SUB],
                            rhs=sup_tiles[kk][:, :],
                            start=(kk == 0),
                            stop=(kk == KT - 1),
                        )
                for s in range(NSUB):
                    g = p * NSUB + s
                    ot = p_out.tile([SUB, OUT_F], F32, tag="ot", name=f"ot{g}")
                    nc.vector.scalar_tensor_tensor(
                        ot[:, :],
                        acc[s][:, :],
                        tsc_sb[0:SUB, 0:1],
                        supsc[g][:, :],
                        mybir.AluOpType.mult,
                        mybir.AluOpType.add,
                    )
                    nc.scalar.dma_start(
                        out=out[g * SUB:(g + 1) * SUB, :], in_=ot[:, :]
                    )

    nc.compile()
    return nc


def _shard_inputs(x, adj, t, weight):
    bf16 = ml_dtypes.bfloat16
    w_bf = np.asarray(weight, np.float32).astype(bf16)
    t0 = float(np.asarray(t, np.float32).reshape(-1)[0])
    tsc = np.empty((128, 2), np.float32)
    tsc[:, 0] = t0
    tsc[:, 1] = 1.0 - t0

    x = np.asarray(x, np.float32)
    adj = np.asarray(adj, np.float32)

    in_maps = []
    for c in range(C):
        rows = slice(c * R, (c + 1) * R)
        adjT = np.ascontiguousarray(adj[rows].T).astype(bf16)   # [N, R]
        adjt = np.ascontiguousarray(
            np.stack([adjT[:, :HALF], adjT[:, HALF:]])          # [2, N, HALF]
        )
        xt = np.ascontiguousarray(x[rows].T).astype(bf16)       # [IN_F, R]
        in_maps.append({"adjt": adjt, "xt": xt, "w": w_bf, "tsc": tsc})
    return in_maps


def kernel(x, adj, t, weight):
    if "nc" not in _cached:
        _cached["nc"] = _build()
    nc = _cached["nc"]
    in_maps = _shard_inputs(x, adj, t, weight)
    res = run_bass_kernel_spmd(nc, in_maps, list(range(C)))
    return np.concatenate([res.results[c]["out"] for c in range(C)], axis=0)


# revision 7
# speedup vs baseline: 1.0583x; 1.0583x over previous
"""AdaptiveDiffusionLayer on 8 TRN2 NeuronCores.

out = (1 - t) * support + t * (adj @ support),  support = x @ weight

Strategy (1D row-parallel SpMM):
  - Row-shard adj and x across 8 cores (1250 rows each); replicate weight/t.
  - Host-side: pre-transpose + bf16-cast each core's adj shard so the
    contraction index k lands on the SBUF partition axis with unit-stride
    DMA (no on-device transposes), packed as [2, N, 625] so each of the
    two PSUM passes reads contiguous slabs.
  - Device: support_c = x_c @ W (bf16 matmul, fp32 PSUM) -> AllGather of
    bf16 support via DRAM bounce -> adj_c @ support accumulated over 79
    k-tiles into 5 PSUM banks per pass (2 passes x 5 i-subtiles of 125
    rows) -> fused epilogue (t * acc + (1-t) * support_c) -> out.
"""

import sys

for _p in ("/opt/trn_rl_repo",):
    if _p not in sys.path:
        sys.path.append(_p)

import numpy as np
import ml_dtypes

from concourse import bass, bacc, mybir, tile
from concourse.bass_utils import run_bass_kernel_spmd

N = 10000
IN_F = 512
OUT_F = 512
C = 8            # cores
R = N // C       # 1250 rows per core
HALF = R // 2    # 625, i-columns per pass
NSUB = 5         # i-subtiles per pass
SUB = HALF // NSUB  # 125 rows per i-subtile
KT = (N + 127) // 128   # 79 k-tiles
KLAST = N - (KT - 1) * 128  # 16

BF16 = mybir.dt.bfloat16
F32 = mybir.dt.float32

_cached = {}


def _build():
    nc = bacc.Bacc("TRN2", target_bir_lowering=False, debug=False, num_devices=C)

    adjt = nc.dram_tensor("adjt", [2, N, HALF], BF16, kind="ExternalInput")
    xt = nc.dram_tensor("xt", [IN_F, R], BF16, kind="ExternalInput")
    w = nc.dram_tensor("w", [IN_F, OUT_F], BF16, kind="ExternalInput")
    tsc = nc.dram_tensor("tsc", [128, 2], F32, kind="ExternalInput")
    out = nc.dram_tensor("out", [R, OUT_F], F32, kind="ExternalOutput")

    sup_in = nc.dram_tensor("sup_in", [R, OUT_F], BF16)
    sup_gath = nc.dram_tensor("sup_gath", [N, OUT_F], BF16, addr_space="Shared")

    NJ = IN_F // 128  # 4 contraction tiles for x @ W

    with tile.TileContext(nc) as tc:
        with (
            tc.tile_pool(name="persist", bufs=1) as p_pers,
            tc.tile_pool(name="psum_sup", bufs=3, space="PSUM") as pp_sup,
            tc.tile_pool(name="supbf_pool", bufs=1) as p_supbf,
            tc.tile_pool(name="sup_pool", bufs=1) as p_sup,
            tc.tile_pool(name="slab_pool", bufs=32) as p_slab,
            tc.tile_pool(name="psum_main", bufs=1, space="PSUM") as pp_main,
            tc.tile_pool(name="out_pool", bufs=4) as p_out,
        ):
            xt_sb = p_pers.tile([128, NJ * R], BF16, tag="xt_sb", name="xt_sb")
            w_sb = p_pers.tile([128, NJ * OUT_F], BF16, tag="w_sb", name="w_sb")
            tsc_sb = p_pers.tile([128, 2], F32, tag="tsc_sb", name="tsc_sb")

            for j in range(NJ):
                nc.scalar.dma_start(
                    out=xt_sb[:, j * R:(j + 1) * R],
                    in_=xt[j * 128:(j + 1) * 128, :],
                )
                nc.scalar.dma_start(
                    out=w_sb[:, j * OUT_F:(j + 1) * OUT_F],
                    in_=w[j * 128:(j + 1) * 128, :],
                )
            nc.scalar.dma_start(out=tsc_sb[:, :], in_=tsc[:, :])

            # ---- support_c = x_c @ W, 10 i-subtiles of 125 rows ----
            supbf = []
            for s in range(2 * NSUB):
                ps = pp_sup.tile([SUB, OUT_F], F32, tag="ps", name=f"ps{s}")
                for j in range(NJ):
                    nc.tensor.matmul(
                        ps[:, :],
                        lhsT=xt_sb[:, j * R + s * SUB: j * R + (s + 1) * SUB],
                        rhs=w_sb[:, j * OUT_F:(j + 1) * OUT_F],
                        start=(j == 0),
                        stop=(j == NJ - 1),
                    )
                sb = p_supbf.tile([SUB, OUT_F], BF16, tag=f"supbf{s}", name=f"supbf{s}")
                nc.vector.tensor_copy(sb[:, :], ps[:, :])
                supbf.append(sb)
                nc.sync.dma_start(
                    out=sup_in[s * SUB:(s + 1) * SUB, :], in_=sb[:, :]
                )

            # ---- AllGather bf16 support across the 8 cores ----
            nc.gpsimd.collective_compute(
                "AllGather",
                mybir.AluOpType.bypass,
                replica_groups=[list(range(C))],
                ins=[sup_in.ap().opt()],
                outs=[sup_gath.ap().opt()],
            )

            # ---- load gathered support as [k-part, f] tiles ----
            sup_tiles = []
            for kk in range(KT):
                sz = 128 if kk < KT - 1 else KLAST
                stile = p_sup.tile([sz, OUT_F], BF16, tag=f"sup{kk}", name=f"sup{kk}")
                nc.scalar.dma_start(
                    out=stile[:, :], in_=sup_gath[kk * 128:kk * 128 + sz, :]
                )
                sup_tiles.append(stile)

            # ---- main SpMM: 2 passes x 5 PSUM accumulators over 79 k-tiles ----
            for p in range(2):
                acc = [
                    pp_main.tile(
                        [SUB, OUT_F], F32, tag=f"acc{s}", name=f"acc{p}_{s}"
                    )
                    for s in range(NSUB)
                ]
                for kk in range(KT):
                    sz = 128 if kk < KT - 1 else KLAST
                    slab = p_slab.tile(
                        [128, HALF], BF16, tag="slab", name=f"slab{p}_{kk}"
                    )
                    nc.sync.dma_start(
                        out=slab[0:sz, :], in_=adjt[p, kk * 128:kk * 128 + sz, :]
                    )
                    for s in range(NSUB):
                        nc.tensor.matmul(
                            acc[s][:, :],
                            lhsT=slab[0:sz, s * SUB:(s + 1) * SUB],
                            rhs=sup_tiles[kk][:, :],
                            start=(kk == 0),
                            stop=(kk == KT - 1),
                        )
                for s in range(NSUB):
                    g = p * NSUB + s
                    sc = p_out.tile([SUB, OUT_F], F32, tag="sc", name=f"osc{g}")
                    nc.vector.tensor_scalar_mul(
                        sc[:, :], supbf[g][:, :], tsc_sb[0:SUB, 1:2]
                    )
                    ot = p_out.tile([SUB, OUT_F], F32, tag="ot", name=f"ot{g}")
                    nc.vector.scalar_tensor_tensor(
                        ot[:, :],
                        acc[s][:, :],
                        tsc_sb[0:SUB, 0:1],
                        sc[:, :],
                        mybir.AluOpType.mult,
                        mybir.AluOpType.add,
                    )
                    nc.scalar.dma_start(
                        out=out[g * SUB:(g + 1) * SUB, :], in_=ot[:, :]
                    )

    nc.compile()
    return nc


def _shard_inputs(x, adj, t, weight):
    bf16 = ml_dtypes.bfloat16
    w_bf = np.asarray(weight, np.float32).astype(bf16)
    t0 = float(np.asarray(t, np.float32).reshape(-1)[0])
    tsc = np.empty((128, 2), np.float32)
    tsc[:, 0] = t0
    tsc[:, 1] = 1.0 - t0

    x = np.asarray(x, np.float32)
    adj = np.asarray(adj, np.float32)

    in_maps = []
    for c in range(C):
        rows = slice(c * R, (c + 1) * R)
        adjT = np.ascontiguousarray(adj[rows].T).astype(bf16)   # [N, R]
        adjt = np.ascontiguousarray(
            np.stack([adjT[:, :HALF], adjT[:, HALF:]])          # [2, N, HALF]
        )
        xt = np.ascontiguousarray(x[rows].T).astype(bf16)       # [IN_F, R]
        in_maps.append({"adjt": adjt, "xt": xt, "w": w_bf, "tsc": tsc})
    return in_maps


def kernel(x, adj, t, weight):
    if "nc" not in _cached:
        _cached["nc"] = _build()
    nc = _cached["nc"]
    in_maps = _shard_inputs(x, adj, t, weight)
    res = run_bass_kernel_spmd(nc, in_maps, list(range(C)))
    return np.concatenate([res.results[c]["out"] for c in range(C)], axis=0)
